# revision 23
# baseline (speedup 1.0000x reference)
import sys

for _p in ("/opt/trn_rl_repo", "/opt/trn_rl_repo/concourse"):
    if _p not in sys.path:
        sys.path.insert(0, _p)

import numpy as np
import ml_dtypes

N_CORES = 8
B, H, W_DIM, C = 8, 32, 32, 288
N = H * W_DIM          # 1024 points per core (batch-dim sharding: 1 image per core)
O = 64                 # codewords
SLAB = 32              # c-slab height; 288 = 9 slabs, zero padding
NSLAB = C // SLAB      # 9
GRP = 4                # o's packed per 128-partition tile (4 x 32)
NGRP = O // GRP        # 16
CHUNK = 512            # PSUM bank free size (fp32)
NCH = N // CHUNK       # 2 chunks
PTBUF = 18             # DVE p-tile ring depth (2+ full groups of 7)
SC_SLABS = (3, 7)      # slabs per group computed on the Scalar engine
SCBUF = 6              # scalar p-tile ring depth (3 groups of 2)
CP3 = 384              # C padded to 3 full 128-partition tiles (for -Sx)

_CACHE = {}
_DEBUG_NAMES = {}


def _patch_drain_split():
    # The end-of-TileContext drain waits on the FULL global clock (PE + DVE
    # + one sem per DMA HW queue), overflowing the CTRL_NO struct's
    # sync-wait slots in walrus. Split: emit one 1-wait SP nop per clock
    # component first; the original drain's full-clock add_sem_waits then
    # elides everything via SP wait history.
    import concourse.tile as tile_mod
    from concourse.vector_clock import ScopedClock, VectorClock

    if getattr(tile_mod.TileContext, "_drain_split_patched", False):
        return

    def _drain_and_barrier(self, tick_clock, wait_clock):
        gc = tick_clock.global_clock
        for idx in range(len(gc)):
            tick = gc[idx]
            if tick <= 0:
                continue
            nop = self.nc.sync.nop(nofuse=True, hint="drain_split")
            vc = VectorClock()
            vc.require_at_least(idx, tick)
            wait_clock.add_sem_waits(nop.ins, ScopedClock({None: vc}))
        # Waitless drain: the nops above (same SP sequencer, in order)
        # already guarantee every sem is at its final value here.
        self.nc.sync.drain()
        self.nc.all_engine_barrier()
        assert self.sems is not None
        popped = self.nc._tile_sem_poison_stack.pop()
        assert popped is self._sem_poison
        self.nc.clear_and_free_semaphores(list(self.sems.allocated().values()))
        self.nc.all_engine_barrier()

    tile_mod.TileContext._drain_and_barrier = _drain_and_barrier
    tile_mod.TileContext._drain_split_patched = True


def _build_program():
    import concourse.bass as bass
    import concourse.tile as tile
    from concourse import mybir

    _patch_drain_split()
    nc = bass.Bass("TRN2", debug=False, num_devices=N_CORES)

    # xrep: slab s (32 c's) replicated 4x across the partition dim, bf16.
    xrep_d = nc.dram_tensor("xrep", [NSLAB * 128, N], mybir.dt.bfloat16, kind="ExternalInput")
    # xt3: plain transposed x, zero-padded to 384 c's (for the -Sx matmuls).
    xt3_d = nc.dram_tensor("xt3", [CP3, N], mybir.dt.bfloat16, kind="ExternalInput")
    # wneg: column 9*g+s = -w packed per (group, slab): [32k+i] = -w[32s+i, 4g+k]
    wneg_d = nc.dram_tensor("wneg", [128, NGRP * NSLAB], mybir.dt.float32, kind="ExternalInput")
    # b2: [p, j] = (b[j] + sum_c w[c,j]) / 128, bf16 (bias via rank-1 matmul)
    b2_d = nc.dram_tensor("b2", [128, O], mybir.dt.bfloat16, kind="ExternalInput")
    out_d = nc.dram_tensor("out_t", [O, N], mybir.dt.float32, kind="ExternalOutput")

    xrep = xrep_d.ap()
    xt3 = xt3_d.ap()
    wneg = wneg_d.ap()
    b2 = b2_d.ap()
    out_t = out_d.ap()

    from contextlib import ExitStack

    from concourse.tile import add_dep_helper

    with tile.TileContext(nc) as tc, ExitStack() as ctx:
        const_pool = ctx.enter_context(tc.tile_pool(name="const", bufs=1))
        psum_pool = ctx.enter_context(tc.tile_pool(name="ps", bufs=1, space="PSUM"))

        # Walrus TensorScalar/Activation ISA structs fit ONE sync wait.
        # Every DMA gets a tiny DVE "touch" so later DVE consumers carry the
        # DMA-queue wait in DVE history; all DVE-sourced deps merge into the
        # single per-engine sem component.
        scratch = const_pool.tile([1, 128], mybir.dt.float32)
        touch_col = [0]

        def touch(src_ap):
            k = touch_col[0]
            touch_col[0] += 1
            bi = nc.vector.tensor_scalar_add(scratch[0:1, k : k + 1], src_ap, 0.0)
            return bi, k

        def touch_write(tile_obj):
            # write into a ring slot, reading only the long-quiet scratch
            # col 127 so the sole fresh wait is the slot's PE reader clock.
            # The write straddles the chunk boundary so it WARs against
            # BOTH chunk matmuls (subtile deps track per-range readers).
            bi = nc.vector.tensor_scalar_add(
                tile_obj[0:1, CHUNK - 1 : CHUNK + 1], scratch[0:1, 126:128], 0.0
            )
            return bi, None

        nc.vector.memset(scratch[:], 0.0)

        # Scalar-engine mirror of the touch machinery (its own scratch and
        # wait history). All scalar ops use Relu so the activation table
        # loads once.
        sscratch = const_pool.tile([1, 128], mybir.dt.float32)
        stouch_col = [0]

        def stouch(src_ap):
            k = stouch_col[0]
            stouch_col[0] += 1
            bi = nc.scalar.activation(
                sscratch[0:1, k : k + 1], src_ap,
                mybir.ActivationFunctionType.Relu, bias=0.0, scale=1.0,
            )
            return bi, k

        def stouch_write(tile_obj):
            bi = nc.scalar.activation(
                tile_obj[0:1, CHUNK - 1 : CHUNK + 1], sscratch[0:1, 126:128],
                mybir.ActivationFunctionType.Relu, bias=0.0, scale=1.0,
            )
            return bi, None

        in_dmas = []

        wneg_sb = const_pool.tile([128, NGRP * NSLAB], mybir.dt.float32)
        in_dmas.append(nc.sync.dma_start(wneg_sb[:], wneg[:, :]))
        touch(wneg_sb[0:1, 0:1])
        # scalar history preload for wneg; also initializes sscratch cols
        # 120..127 (stouch_write reads 126:128)
        nc.scalar.activation(
            sscratch[0:1, 120:128], wneg_sb[0:1, 0:8],
            mybir.ActivationFunctionType.Relu, bias=0.0, scale=1.0,
        )

        x_sb = []
        for s in range(NSLAB):
            xs = const_pool.tile([128, N], mybir.dt.bfloat16, name=f"x_sb{s}")
            in_dmas.append(nc.sync.dma_start(xs[:], xrep[128 * s : 128 * (s + 1), :]))
            touch(xs[0:1, 0:1])
            if s in SC_SLABS:
                stouch(xs[0:1, 0:1])
            x_sb.append(xs)

        b2_sb = const_pool.tile([128, O], mybir.dt.bfloat16)
        in_dmas.append(nc.sync.dma_start(b2_sb[:], b2[:, :]))
        touch(b2_sb[0:1, 0:1])

        xt3_sb = []
        for t in range(3):
            xs = const_pool.tile([128, N], mybir.dt.bfloat16, name=f"xt3_{t}")
            in_dmas.append(nc.sync.dma_start(xs[:], xt3[128 * t : 128 * (t + 1), :]))
            touch(xs[0:1, 0:1])
            xt3_sb.append(xs)

        # zwin[p, 64 + p//32] = 2.0 else 0. lhsT for group g = zwin[:, 64-4g :
        # 128-4g]: window column j holds the 2.0-block for output partition j
        # exactly when j = 4g + p//32 — routes 2*sum_c(relu) of o-block k
        # onto PSUM partition 4g+k.
        zwin = const_pool.tile([128, 128], mybir.dt.bfloat16)
        nc.vector.memset(zwin[:], 0.0)
        for k in range(GRP):
            nc.vector.memset(zwin[32 * k : 32 * (k + 1), 64 + k : 65 + k], 2.0)

        # all-(-1) lhsT: -Sx[n] accumulated onto every output partition
        neg1 = const_pool.tile([128, O], mybir.dt.bfloat16)
        nc.vector.memset(neg1[:], -1.0)
        # all-ones rhs for the rank-1 bias matmul
        ones = const_pool.tile([128, CHUNK], mybir.dt.bfloat16)
        nc.vector.memset(ones[:], 1.0)

        ps = [
            psum_pool.tile([O, CHUNK], mybir.dt.float32, name=f"ps{ch}")
            for ch in range(NCH)
        ]

        # p-tile rings: 18 fixed DVE tiles (7 per group -> ~2.5 groups deep)
        # and 6 scalar tiles (2 per group -> 3 groups deep). A writer
        # rewriting slot j-RING carries a WAR against that slot's old PE
        # readers and a WAW against its old writer; both are pre-absorbed
        # into the writing engine's wait history once per group (rt/at/wt)
        # so each real producer op carries <=1 sync wait (walrus TS/ACT
        # ISA structs fit only one).
        DVE_PER_GRP = NSLAB - len(SC_SLABS)  # 7
        pt = [
            const_pool.tile([128, N], mybir.dt.bfloat16, name=f"pt{j}")
            for j in range(PTBUF)
        ]
        spt = [
            const_pool.tile([128, N], mybir.dt.bfloat16, name=f"spt{j}")
            for j in range(SCBUF)
        ]
        dve_hist = []  # dve tile index -> ring tile
        sc_hist = []

        # out[o, n] = 2*sum_c relu(x-w) - Sx[n] + Sw[o] + b[o]
        for g in range(NGRP):
            wt = None
            swt = None
            jt = DVE_PER_GRP * g + DVE_PER_GRP - 1 - PTBUF
            if jt >= 0:
                # newest ring slot group g will overwrite: its old writer /
                # PE readers dominate every other slot the group touches.
                jt = min(jt, len(dve_hist) - 1)
                rt, krt = touch(dve_hist[jt][0:1, 0:1])
                at, _ = touch(scratch[0:1, krt : krt + 1])
                wt, _ = touch_write(dve_hist[jt])
                _DEBUG_NAMES[rt.ins.name] = f"rt{g}"
                _DEBUG_NAMES[at.ins.name] = f"at{g}"
                _DEBUG_NAMES[wt.ins.name] = f"wt{g}"
            sjt = len(SC_SLABS) * g + len(SC_SLABS) - 1 - SCBUF
            if sjt >= 0:
                sjt = min(sjt, len(sc_hist) - 1)
                srt, skrt = stouch(sc_hist[sjt][0:1, 0:1])
                sat, _ = stouch(sscratch[0:1, skrt : skrt + 1])
                swt, _ = stouch_write(sc_hist[sjt])
                _DEBUG_NAMES[srt.ins.name] = f"srt{g}"
                _DEBUG_NAMES[sat.ins.name] = f"sat{g}"
                _DEBUG_NAMES[swt.ins.name] = f"swt{g}"

            for s in range(NSLAB):
                i = NSLAB * g + s
                col = i
                if s in SC_SLABS:
                    p = spt[len(sc_hist) % SCBUF]
                    sc_hist.append(p)
                    pr = nc.scalar.activation(
                        p[:], x_sb[s][:],
                        mybir.ActivationFunctionType.Relu,
                        bias=wneg_sb[:, col : col + 1], scale=1.0,
                    )
                    gate = swt
                else:
                    p = pt[len(dve_hist) % PTBUF]
                    dve_hist.append(p)
                    pr = nc.vector.tensor_scalar(
                        p[:], x_sb[s][:], wneg_sb[:, col : col + 1], 0.0,
                        op0=mybir.AluOpType.add,
                        op1=mybir.AluOpType.max,
                    )
                    gate = wt
                _DEBUG_NAMES[pr.ins.name] = f"p{i}"
                if gate is not None:
                    # scheduling-only edge: keep every producer of this
                    # group after the group's absorber, so the PE wait is
                    # already in the engine's history when it is placed.
                    add_dep_helper(pr.ins, gate.ins, sync=False,
                                   reason="producer after group absorber")
                for ch in range(NCH):
                    nc.tensor.matmul(
                        ps[ch][:],
                        lhsT=zwin[:, 64 - 4 * g : 128 - 4 * g],
                        rhs=p[:, CHUNK * ch : CHUNK * (ch + 1)],
                        start=(g == 0 and s == 0),
                        stop=False,
                    )

        # -Sx[n]: sum over all c (3 padded 128-tiles) with weight -1
        for t in range(3):
            for ch in range(NCH):
                nc.tensor.matmul(
                    ps[ch][:],
                    lhsT=neg1[:, 0:O],
                    rhs=xt3_sb[t][:, CHUNK * ch : CHUNK * (ch + 1)],
                    start=False,
                    stop=False,
                )
        # + (b[o] + Sw[o]): rank-1 matmul, lhsT column j = (b[j]+Sw[j])/128
        for ch in range(NCH):
            nc.tensor.matmul(
                ps[ch][:],
                lhsT=b2_sb[:, 0:O],
                rhs=ones[:, :],
                start=False,
                stop=True,
            )

        out_sb = const_pool.tile([O, N], mybir.dt.float32)
        for ch in range(NCH):
            nc.vector.tensor_scalar_add(
                out_sb[:, CHUNK * ch : CHUNK * (ch + 1)], ps[ch][:], 0.0
            )
        # A HWDGE out-DMA descriptor fits one sync wait, but the hardware
        # queue also forces a ring-ordering wait behind the input DMAs --
        # two waits, which walrus can't encode. GpSimd issues DMAs in
        # software with no such limit.
        nc.gpsimd.dma_start(out_t[:, :], out_sb[:])

    return nc


def _prep_inputs(x, w, b):
    xs = x.reshape(B, N, C).astype(np.float32)
    wf = w.astype(np.float32)

    # wneg[32k+i, 9g+s] = -w[32s+i, 4g+k]
    wneg = np.empty((128, NGRP * NSLAB), dtype=np.float32)
    for g in range(NGRP):
        for s in range(NSLAB):
            blk = -wf[SLAB * s : SLAB * (s + 1), GRP * g : GRP * (g + 1)]  # [32, 4]
            wneg[:, NSLAB * g + s] = blk.T.reshape(128)

    sw = wf.sum(axis=0, dtype=np.float64)
    b2row = ((b.astype(np.float64) + sw) / 128.0).astype(ml_dtypes.bfloat16)
    b2 = np.broadcast_to(b2row, (128, O)).copy()

    in_maps = []
    for core in range(N_CORES):
        xt = xs[core].T.astype(ml_dtypes.bfloat16)  # [288, 1024]
        xrep = np.empty((NSLAB * 128, N), dtype=ml_dtypes.bfloat16)
        for s in range(NSLAB):
            slab = xt[SLAB * s : SLAB * (s + 1), :]  # [32, 1024]
            xrep[128 * s : 128 * (s + 1), :] = np.tile(slab, (GRP, 1))
        xt3 = np.zeros((CP3, N), dtype=ml_dtypes.bfloat16)
        xt3[:C, :] = xt
        in_maps.append({"xrep": xrep, "xt3": xt3, "wneg": wneg, "b2": b2})
    return in_maps


def kernel(x, w, b):
    from concourse.bass_utils import run_bass_kernel_spmd

    if "nc" not in _CACHE:
        _CACHE["nc"] = _build_program()
    nc = _CACHE["nc"]

    in_maps = _prep_inputs(x, w, b)
    res = run_bass_kernel_spmd(nc, in_maps, list(range(N_CORES)))
    out = np.stack(
        [np.asarray(res.results[core]["out_t"], dtype=np.float32).T for core in range(N_CORES)]
    )
    return out.astype(np.float32)
